# revision 20
# baseline (speedup 1.0000x reference)
"""TSSA causal self-attention Bass kernel for 8 TRN2 NeuronCores.

Math (per batch b):
    w      = x @ Wa.T + ba                  # (T, C) -> heads (H, T, D)
    wsq    = w * w
    denom  = cumsum_T(wsq)                  # inclusive
    tmp    = (sum_d(wsq / denom) + D*db) * temp          # (T, H)
    Pi     = softmax_h(tmp)                 # (T, H)
    cumA   = cumsum_T(wsq * Pi)
    cumPi  = cumsum_T(Pi) + 1e-8
    y      = -(w * Pi) * (1 / (1 + cumA / cumPi))
           = (w * Pi * cumPi) * (-1 / (cumA + cumPi))
    out    = y @ Wp.T + bp

Sharding: core i -> (batch b = i//2, T-half = i%2).  Each core runs the full
pipeline on its (b, T/2) slice in [t-on-partitions, c-free] layout, chunked by
128 t-rows.  Cumsums over T are triangular matmuls on the PE plus per-chunk
carry rows; the cross-half carries travel via two tiny pairwise AllGathers.
"""

import numpy as np
import ml_dtypes

B, T, C, H, D = 4, 4096, 1024, 16, 64
N_CORES = 8
P = 128
T_LOCAL = T // 2

F32 = None  # filled on bass import
BF16 = None

_BUILD_CACHE = {}


def _build(t_local, n_groups, use_bf16=True):
    """Build the SPMD Bass program. n_groups = number of core pairs."""
    import concourse.bass as bass
    import concourse.bacc as bacc
    import concourse.mybir as mybir
    from concourse import tile

    dt = mybir.dt
    f32, bf16, f32r = dt.float32, dt.bfloat16, dt.float32r
    AF = mybir.ActivationFunctionType
    OP = mybir.AluOpType

    n_chunks = t_local // P
    n_cores = 2 * n_groups
    NH = n_chunks  # alias

    wdt = bf16 if use_bf16 else f32  # dtype for wsp store + proj2 path
    npwdt = ml_dtypes.bfloat16 if use_bf16 else np.float32

    nc = bacc.Bacc(None, target_bir_lowering=False, debug=False)

    # ---------------- I/O ----------------
    xT = nc.dram_tensor("xT", [C, t_local], f32r, kind="ExternalInput")
    waT = nc.dram_tensor("waT", [C, C], f32r, kind="ExternalInput")
    wpTn = nc.dram_tensor("wpTn", [C, C], wdt, kind="ExternalInput")
    ba_in = nc.dram_tensor("ba", [1, C], f32r, kind="ExternalInput")
    onesr_in = nc.dram_tensor("onesr", [1, P], f32r, kind="ExternalInput")
    onesb_in = nc.dram_tensor("onesb", [1, P], wdt, kind="ExternalInput")
    bp_in = nc.dram_tensor("bp", [1, C], wdt, kind="ExternalInput")
    tb_in = nc.dram_tensor("tb", [P, H], f32, kind="ExternalInput")
    db_in = nc.dram_tensor("db64", [t_local, H], f32, kind="ExternalInput")
    pfx_in = nc.dram_tensor("pfx", [NH + 2, NH], f32, kind="ExternalInput")
    out = nc.dram_tensor("out", [t_local, C], f32, kind="ExternalOutput")

    # constants baked into the NEFF
    ut_np = np.triu(np.ones((P, P), np.float32))
    ut_c = nc.inline_tensor(ut_np, "ut_c")
    utb_c = nc.inline_tensor(ut_np.astype(npwdt), "utb_c")
    ones_np = np.ones((P, P), np.float32)
    ones_c = nc.inline_tensor(ones_np, "ones_c")
    eye_np = np.eye(P, dtype=np.float32)
    eye_c = nc.inline_tensor(eye_np, "eye_c")
    eyeb_c = nc.inline_tensor(eye_np.astype(npwdt), "eyeb_c")
    bm_np = np.zeros((H, C), np.float32)
    for h in range(H):
        bm_np[h, h * D:(h + 1) * D] = 1.0
    bm_c = nc.inline_tensor(bm_np, "bm_c")
    bmb_c = nc.inline_tensor(bm_np.astype(npwdt), "bmb_c")
    # one-hot chunk selectors: oneh[:, j, m] = (m == j)
    oneh_np = np.zeros((P, NH, NH), np.float32)
    for j in range(NH):
        oneh_np[:, j, j] = 1.0
    oneh_c = nc.inline_tensor(oneh_np.reshape(P, NH * NH), "oneh_c")
    onehb_c = nc.inline_tensor(oneh_np.reshape(P, NH * NH).astype(npwdt),
                               "onehb_c")

    # internal DRAM for collectives
    cc1_in = nc.dram_tensor("cc1_in", [1, C], f32, kind="Internal")
    cc1_out = nc.dram_tensor("cc1_out", [2, C], f32, kind="Internal")
    cc2_in = nc.dram_tensor("cc2_in", [1, C + H], f32, kind="Internal")
    cc2_out = nc.dram_tensor("cc2_out", [2, C + H], f32, kind="Internal")
    carr_d = nc.dram_tensor("carr_d", [t_local // P, C], f32, kind="Internal")
    carrA_d = nc.dram_tensor("carrA_d", [t_local // P, C], wdt, kind="Internal")
    carrPi_d = nc.dram_tensor("carrPi_d", [t_local // P, H], f32, kind="Internal")
    rg1 = [[2 * g, 2 * g + 1] for g in range(n_groups)]

    def r(ap):
        return ap.bitcast(f32r)

    with tile.TileContext(nc) as tc:
        with (
            tc.tile_pool(name="const", bufs=1) as cpool,
            tc.tile_pool(name="persist", bufs=1) as pp,
            tc.tile_pool(name="wmat", bufs=1) as wm,
        ):
            # ---- constants to SBUF ----
            ut_s = cpool.tile([P, P], f32, tag="ut")
            nc.sync.dma_start(ut_s[:, :], ut_c.ap())
            utb_s = cpool.tile([P, P], wdt, tag="utb")
            nc.sync.dma_start(utb_s[:, :], utb_c.ap())
            ones_s = cpool.tile([P, P], f32, tag="ones")
            nc.sync.dma_start(ones_s[:, :], ones_c.ap())
            eye_s = cpool.tile([P, P], f32, tag="eye")
            nc.sync.dma_start(eye_s[:, :], eye_c.ap())
            eyeb_s = cpool.tile([P, P], wdt, tag="eyeb")
            nc.sync.dma_start(eyeb_s[:, :], eyeb_c.ap())
            bmb_s = cpool.tile([H, C], wdt, tag="bmb")
            nc.sync.dma_start(bmb_s[:, :], bmb_c.ap())
            ba_s = cpool.tile([1, C], f32r, tag="ba")
            nc.sync.dma_start(ba_s[:, :], ba_in.ap())
            bp_s = cpool.tile([1, C], wdt, tag="bp")
            nc.sync.dma_start(bp_s[:, :], bp_in.ap())
            tb_s = cpool.tile([P, H], f32, tag="tb")
            nc.sync.dma_start(tb_s[:, :], tb_in.ap())
            onesr_s = cpool.tile([1, P], f32r, tag="onesr")
            nc.sync.dma_start(onesr_s[:, :], onesr_in.ap())
            onesb_s = cpool.tile([1, P], wdt, tag="onesb")
            nc.sync.dma_start(onesb_s[:, :], onesb_in.ap())
            db_s = cpool.tile([P, NH, H], f32, tag="db")
            nc.sync.dma_start(db_s[:, :, :],
                              db_in.ap().rearrange("(j p) h -> p j h", p=P))
            pfx_s = cpool.tile([NH + 2, NH], f32, tag="pfx")
            nc.sync.dma_start(pfx_s[:, :], pfx_in.ap())
            oneh_s = cpool.tile([P, NH, NH], f32, tag="oneh")
            nc.sync.dma_start(
                oneh_s[:, :, :],
                oneh_c.ap().rearrange("p (j m) -> p j m", j=NH))
            onehb_s = cpool.tile([P, NH, NH], wdt, tag="onehb")
            nc.sync.dma_start(
                onehb_s[:, :, :],
                onehb_c.ap().rearrange("p (j m) -> p j m", j=NH))

            # ---- persistent stores ----
            w_st = pp.tile([P, NH, C], f32, tag="w_st")
            wsp_st = pp.tile([P, NH, C], wdt, tag="wsp_st")
            pi_st = pp.tile([P, NH, H], f32, tag="pi_st")
            tmp_st = pp.tile([P, NH, H], f32, tag="tmp_st")
            nm_st = pp.tile([P, NH], f32, tag="nm_st")
            s_tbl = pp.tile([NH + 2, C], f32, tag="s_tbl")
            swsp_tbl = pp.tile([NH + 2, C], f32, tag="swsp_tbl")
            spi_tbl = pp.tile([NH + 2, H], f32, tag="spi_tbl")

            wa_s = wm.tile([P, 8, C], f32r, tag="wa")
            nc.sync.dma_start(wa_s[:, :, :],
                              waT.ap().rearrange("(a p) n -> p a n", p=P))

            xT_r = xT.ap().rearrange("(a p) t -> p a t", p=P)

            # ================= phase 1: proj1, store w, S_wsq rows ========
            with (
                tc.tile_pool(name="xt", bufs=3) as xt_pool,
                tc.tile_pool(name="sq1", bufs=2) as sq_pool,
                tc.tile_pool(name="pw", bufs=2, space="PSUM") as pw_pool,
                tc.tile_pool(name="pS", bufs=1, space="PSUM") as pS_pool,
            ):
                pS = pS_pool.tile([NH, C], f32, tag="pS")
                for j in range(n_chunks):
                    xt = xt_pool.tile([P, 8, P], f32r, tag="xt")
                    nc.sync.dma_start(xt[:, :, :], xT_r[:, :, j * P:(j + 1) * P])
                    pw = pw_pool.tile([P, C], f32, tag="pw")
                    for hh in range(2):
                        o = pw[:, hh * 512:(hh + 1) * 512]
                        for a in range(8):
                            nc.tensor.matmul(
                                o, xt[:, a, :],
                                wa_s[:, a, hh * 512:(hh + 1) * 512],
                                start=(a == 0), stop=False)
                        nc.tensor.matmul(
                            o, onesr_s[0:1, :],
                            ba_s[0:1, hh * 512:(hh + 1) * 512],
                            start=False, stop=True)
                    # drain: w copy (DVE) + square (ACT)
                    nc.vector.tensor_copy(w_st[:, j, :], pw[:, :])
                    sq = sq_pool.tile([P, C], wdt, tag="sq1")
                    nc.scalar.activation(sq[:, :], pw[:, :], AF.Square)
                    # S_wsq row j (one-hot selector accumulates into row j)
                    for hh in range(2):
                        nc.tensor.matmul(
                            pS[0:NH, hh * 512:(hh + 1) * 512],
                            onehb_s[:, j, :],
                            sq[:, hh * 512:(hh + 1) * 512],
                            start=(j == 0), stop=(j == n_chunks - 1))
                # export S rows + total
                nc.vector.tensor_copy(s_tbl[0:NH, :], pS[:, :])
                pt = pw_pool.tile([P, C], f32, tag="pw")
                for hh in range(2):
                    nc.tensor.matmul(pt[0:1, hh * 512:(hh + 1) * 512],
                                     ones_s[0:NH, 0:1],
                                     s_tbl[0:NH, hh * 512:(hh + 1) * 512],
                                     start=True, stop=True)
                cc1_stage = sq_pool.tile([1, C], f32, tag="cc1s")
                nc.vector.tensor_copy(cc1_stage[:, :], pt[0:1, :])
                nc.sync.dma_start(cc1_in.ap(), cc1_stage[:, :])

            # ---- collective 1 + denom carry table ----
            nc.gpsimd.collective_compute(
                "AllGather", mybir.AluOpType.bypass,
                replica_groups=rg1,
                ins=[cc1_in.ap().opt()],
                outs=[cc1_out.ap().opt()])
            nc.sync.dma_start(s_tbl[NH:NH + 2, :], cc1_out.ap())

            with (
                tc.tile_pool(name="pc", bufs=1, space="PSUM") as pc_pool,
                tc.tile_pool(name="pcs", bufs=1) as pcs_pool,
            ):
                pc = pc_pool.tile([NH, C], f32, tag="pc")
                for hh in range(2):
                    nc.tensor.matmul(pc[:, hh * 512:(hh + 1) * 512],
                                     pfx_s[:, :],
                                     s_tbl[:, hh * 512:(hh + 1) * 512],
                                     start=True, stop=True)
                pcs = pcs_pool.tile([NH, C], f32, tag="pcs")
                nc.vector.tensor_copy(pcs[:, :], pc[:, :])
                nc.sync.dma_start(carr_d.ap(), pcs[:, :])

            # ============ phase 2a: denom, recip, tmp ====================
            with (
                tc.tile_pool(name="sq2", bufs=2) as sq_pool,
                tc.tile_pool(name="rd2", bufs=2) as rd_pool,
                tc.tile_pool(name="wn2", bufs=2) as wn_pool,
                tc.tile_pool(name="t2a", bufs=2) as tiny_pool,
                tc.tile_pool(name="ct2", bufs=3) as ct_pool,
                tc.tile_pool(name="pd", bufs=2, space="PSUM") as pd_pool,
            ):
                for j in range(n_chunks):
                    sq = sq_pool.tile([P, C], f32, tag="sq2")
                    nc.scalar.activation(sq[:, :], w_st[:, j, :], AF.Square)
                    ct = ct_pool.tile([1, C], f32, tag="ct2")
                    nc.sync.dma_start(ct[0:1, :], carr_d.ap()[j:j + 1, :])
                    pd = pd_pool.tile([P, C], f32, tag="pd")
                    for hh in range(2):
                        o = pd[:, hh * 512:(hh + 1) * 512]
                        nc.tensor.matmul(o, ut_s[:, :],
                                         sq[:, hh * 512:(hh + 1) * 512],
                                         start=True, stop=False)
                        nc.tensor.matmul(o, ones_s[0:1, :],
                                         ct[0:1, hh * 512:(hh + 1) * 512],
                                         start=False, stop=True)
                    rd = rd_pool.tile([P, C], f32, tag="rd2")
                    nc.vector.reciprocal_approx_fast(rd[:, :], pd[:, :])
                    wn = wn_pool.tile([P, C], f32, tag="wn2")
                    nc.vector.tensor_mul(wn[:, :], sq[:, :], rd[:, :])
                    red = tiny_pool.tile([P, H], f32, tag="red")
                    nc.vector.tensor_reduce(
                        red[:, :], wn[:, :].rearrange("p (h d) -> p h d", d=D),
                        axis=mybir.AxisListType.X, op=OP.add)
                    t1 = tiny_pool.tile([P, H], f32, tag="t1")
                    nc.vector.tensor_add(t1[:, :], red[:, :], db_s[:, j, :])
                    nc.vector.tensor_mul(tmp_st[:, j, :], t1[:, :], tb_s[:, :])
                    nc.vector.tensor_reduce(
                        nm_st[:, j:j + 1], tmp_st[:, j, :],
                        axis=mybir.AxisListType.X, op=OP.max, negate=True)

            # ============ phase 2b: batched softmax over heads ===========
            with tc.tile_pool(name="sm", bufs=1) as sm_pool:
                sh = sm_pool.tile([P, NH, H], f32, tag="sh")
                nc.vector.tensor_add(
                    sh[:, :, :], tmp_st[:, :, :],
                    nm_st[:, :].rearrange("p (j o) -> p j o", o=1).to_broadcast((P, NH, H)))
                es = sm_pool.tile([P, NH, H], f32, tag="es")
                nc.scalar.activation(es[:, :, :], sh[:, :, :], AF.Exp)
                rs = sm_pool.tile([P, NH], f32, tag="rs")
                nc.vector.tensor_reduce(rs[:, :], es[:, :, :],
                                        axis=mybir.AxisListType.X, op=OP.add)
                rr = sm_pool.tile([P, NH], f32, tag="rr")
                nc.vector.reciprocal(rr[:, :], rs[:, :])
                nc.vector.tensor_mul(
                    pi_st[:, :, :], es[:, :, :],
                    rr[:, :].rearrange("p (j o) -> p j o", o=1).to_broadcast((P, NH, H)))

            # ============ phase 2c: wsp + S_pi / S_wsp rows ==============
            with (
                tc.tile_pool(name="sq3", bufs=2) as sq_pool,
                tc.tile_pool(name="pSP", bufs=1, space="PSUM") as pSP_pool,
                tc.tile_pool(name="pSW", bufs=1, space="PSUM") as pSW_pool,
            ):
                pSPI = pSP_pool.tile([NH, H], f32, tag="pSPI")
                pSW = pSW_pool.tile([NH, C], f32, tag="pSW")
                for j in range(n_chunks):
                    sq = sq_pool.tile([P, C], f32, tag="sq3")
                    nc.scalar.activation(sq[:, :], w_st[:, j, :], AF.Square)
                    nc.vector.tensor_mul(
                        wsp_st[:, j, :], sq[:, :],
                        pi_st[:, j, :].rearrange("p (h o) -> p h o", o=1)
                        .to_broadcast((P, H, D)))
                    nc.tensor.matmul(pSPI[0:NH, :], oneh_s[:, j, :],
                                     pi_st[:, j, :],
                                     start=(j == 0), stop=(j == n_chunks - 1))
                    for hh in range(2):
                        nc.tensor.matmul(
                            pSW[0:NH, hh * 512:(hh + 1) * 512],
                            onehb_s[:, j, :],
                            wsp_st[:, j, hh * 512:(hh + 1) * 512],
                            start=(j == 0), stop=(j == n_chunks - 1))
                nc.vector.tensor_copy(swsp_tbl[0:NH, :], pSW[:, :])
                nc.vector.tensor_copy(spi_tbl[0:NH, :], pSPI[:, :])
                # totals
                pt2 = pSW_pool.tile([NH, C], f32, tag="pSW")
                for hh in range(2):
                    nc.tensor.matmul(pt2[0:1, hh * 512:(hh + 1) * 512],
                                     ones_s[0:NH, 0:1],
                                     swsp_tbl[0:NH, hh * 512:(hh + 1) * 512],
                                     start=True, stop=True)
                pt3 = pSP_pool.tile([NH, H], f32, tag="pSPI")
                nc.tensor.matmul(pt3[0:1, 0:H], ones_s[0:NH, 0:1],
                                 spi_tbl[0:NH, :], start=True, stop=True)
                cc2_stage = sq_pool.tile([1, C + H], f32, tag="cc2s")
                nc.vector.tensor_copy(cc2_stage[:, 0:C], pt2[0:1, :])
                nc.vector.tensor_copy(cc2_stage[:, C:C + H], pt3[0:1, 0:H])
                nc.sync.dma_start(cc2_in.ap(), cc2_stage[:, :])

            # ---- collective 2 + dots carry tables ----
            nc.gpsimd.collective_compute(
                "AllGather", mybir.AluOpType.bypass,
                replica_groups=rg1,
                ins=[cc2_in.ap().opt()],
                outs=[cc2_out.ap().opt()])
            nc.sync.dma_start(swsp_tbl[NH:NH + 2, :], cc2_out.ap()[:, 0:C])
            nc.sync.dma_start(spi_tbl[NH:NH + 2, :], cc2_out.ap()[:, C:C + H])

            with (
                tc.tile_pool(name="pc2", bufs=1, space="PSUM") as pc_pool,
                tc.tile_pool(name="pcs2", bufs=1) as pcs_pool,
            ):
                pc = pc_pool.tile([NH, C], f32, tag="pc2")
                for hh in range(2):
                    nc.tensor.matmul(pc[:, hh * 512:(hh + 1) * 512],
                                     pfx_s[:, :],
                                     swsp_tbl[:, hh * 512:(hh + 1) * 512],
                                     start=True, stop=True)
                pcs = pcs_pool.tile([NH, C], wdt, tag="pcs2")
                nc.vector.tensor_copy(pcs[:, :], pc[:, :])
                nc.sync.dma_start(carrA_d.ap(), pcs[:, :])
                pc3 = pc_pool.tile([NH, C], f32, tag="pc2")
                nc.tensor.matmul(pc3[:, 0:H], pfx_s[:, :], spi_tbl[:, :],
                                 start=True, stop=True)
                pcs3 = pcs_pool.tile([NH, H], f32, tag="pcs3")
                nc.vector.tensor_copy(pcs3[:, :], pc3[:, 0:H])
                nc.sync.dma_start(carrPi_d.ap(), pcs3[:, :])

            # ================= phase 3: dots, attn, y, proj2 =============
            wp_s = wm.tile([P, 8, C], wdt, tag="wa")
            nc.sync.dma_start(wp_s[:, :, :],
                              wpTn.ap().rearrange("(a p) n -> p a n", p=P))
            with (
                tc.tile_pool(name="t3", bufs=2) as tiny_pool,
                tc.tile_pool(name="rd3", bufs=2) as rd_pool,
                tc.tile_pool(name="yy", bufs=2) as y_pool,
                tc.tile_pool(name="yt", bufs=2) as yt_pool,
                tc.tile_pool(name="ost", bufs=2) as o_pool,
                tc.tile_pool(name="psm", bufs=1, space="PSUM") as psm_pool,
                tc.tile_pool(name="pD", bufs=2, space="PSUM") as pD_pool,
                tc.tile_pool(name="pyt", bufs=1, space="PSUM") as pyt_pool,
                tc.tile_pool(name="po", bufs=1, space="PSUM") as po_pool,
            ):
                for j in range(n_chunks):
                    # cumPi for this chunk
                    ctp = tiny_pool.tile([1, H], f32, tag="ctp")
                    nc.sync.dma_start(ctp[0:1, :], carrPi_d.ap()[j:j + 1, :])
                    cta = tiny_pool.tile([1, C], wdt, tag="cta")
                    nc.sync.dma_start(cta[0:1, :], carrA_d.ap()[j:j + 1, :])
                    ps = psm_pool.tile([P, 512], f32, tag="psm")
                    nc.tensor.matmul(ps[:, 0:H], ut_s[:, :], pi_st[:, j, :],
                                     start=True, stop=False)
                    nc.tensor.matmul(ps[:, 0:H], ones_s[0:1, :],
                                     ctp[0:1, :],
                                     start=False, stop=True)
                    cpe = tiny_pool.tile([P, H], f32, tag="cpe")
                    nc.vector.tensor_scalar_add(cpe[:, :], ps[:, 0:H], 1e-8)
                    # transpose cpe -> [H, P]
                    nc.tensor.transpose(ps[0:H, 128:256], cpe[:, :],
                                        eye_s[:, :])
                    cpt = tiny_pool.tile([H, P], wdt, tag="cpt")
                    nc.vector.tensor_copy(cpt[:, :], ps[0:H, 128:256])
                    # D = cumA + bcast(cumPi_e)
                    pD = pD_pool.tile([P, C], f32, tag="pD")
                    for hh in range(2):
                        o = pD[:, hh * 512:(hh + 1) * 512]
                        nc.tensor.matmul(o, utb_s[:, :],
                                         wsp_st[:, j, hh * 512:(hh + 1) * 512],
                                         start=True, stop=False)
                        nc.tensor.matmul(o, onesb_s[0:1, :],
                                         cta[0:1, hh * 512:(hh + 1) * 512],
                                         start=False, stop=False)
                        nc.tensor.matmul(o, cpt[:, :],
                                         bmb_s[:, hh * 512:(hh + 1) * 512],
                                         start=False, stop=True)
                    rd = rd_pool.tile([P, C], f32, tag="rd3")
                    nc.vector.reciprocal_approx_fast(rd[:, :], pD[:, :])
                    # g = Pi * cumPi_e
                    g = tiny_pool.tile([P, H], f32, tag="g")
                    nc.vector.tensor_mul(g[:, :], pi_st[:, j, :], cpe[:, :])
                    # y = (w * rd) * g_bcast   (positive; sign folded into wpTn)
                    t1 = y_pool.tile([P, C], f32, tag="t1f")
                    nc.vector.tensor_mul(t1[:, :], w_st[:, j, :], rd[:, :])
                    y = y_pool.tile([P, C], wdt, tag="ybf")
                    nc.vector.tensor_mul(
                        y[:, :], t1[:, :],
                        g[:, :].rearrange("p (h o) -> p h o", o=1).to_broadcast((P, H, D)))
                    # transpose y -> yT (8 PE transposes via 2 psum rounds)
                    yt = yt_pool.tile([P, 8, P], wdt, tag="yt")
                    for rnd in range(2):
                        pyt = pyt_pool.tile([P, 512], wdt, tag="pyt")
                        for i in range(4):
                            a = rnd * 4 + i
                            nc.tensor.transpose(
                                pyt[:, i * P:(i + 1) * P],
                                y[:, a * P:(a + 1) * P], eyeb_s[:, :])
                        nc.scalar.copy(yt[:, rnd * 4:rnd * 4 + 4, :],
                                       pyt[:, :])
                    # proj2
                    po = po_pool.tile([P, C], f32, tag="po")
                    for hh in range(2):
                        o = po[:, hh * 512:(hh + 1) * 512]
                        for a in range(8):
                            nc.tensor.matmul(
                                o, yt[:, a, :],
                                wp_s[:, a, hh * 512:(hh + 1) * 512],
                                start=(a == 0), stop=False)
                        nc.tensor.matmul(o, onesb_s[0:1, :],
                                         bp_s[0:1, hh * 512:(hh + 1) * 512],
                                         start=False, stop=True)
                    for hh in range(2):
                        ost = o_pool.tile([P, 512], f32, tag="ost")
                        nc.vector.tensor_copy(ost[:, :],
                                              po[:, hh * 512:(hh + 1) * 512])
                        nc.sync.dma_start(
                            out.ap()[j * P:(j + 1) * P,
                                     hh * 512:(hh + 1) * 512], ost[:, :])

    nc.finalize()
    return nc


def _get_nc(t_local=T_LOCAL, n_groups=4, use_bf16=True):
    key = (t_local, n_groups, use_bf16)
    if key not in _BUILD_CACHE:
        _BUILD_CACHE[key] = _build(t_local, n_groups, use_bf16)
    return _BUILD_CACHE[key]


def make_in_maps(x, Wa, ba, Wp, bp, temp, denom_bias, t_local=T_LOCAL,
                 n_groups=4, use_bf16=True):
    """Host-side sharding: core i -> (b=i//2, half=i%2)."""
    npwdt = ml_dtypes.bfloat16 if use_bf16 else np.float32
    n_chunks = t_local // P
    waT = np.ascontiguousarray(Wa.T.astype(np.float32))
    onesr = np.ones((1, P), np.float32)
    onesb = np.ones((1, P), npwdt)
    wpTn = np.ascontiguousarray((-Wp.T).astype(np.float32)).astype(npwdt)
    ba_r = np.ascontiguousarray(ba.reshape(1, C).astype(np.float32))
    bp_r = np.ascontiguousarray(bp.reshape(1, C).astype(np.float32)).astype(npwdt)
    tb = np.broadcast_to(temp.reshape(1, H), (P, H)).astype(np.float32)
    tb = np.ascontiguousarray(tb)
    in_maps = []
    for i in range(2 * n_groups):
        b, half = i // 2, i % 2
        t0 = half * t_local
        xT = np.ascontiguousarray(x[b, t0:t0 + t_local, :].T.astype(np.float32))
        db64 = np.ascontiguousarray(
            (D * denom_bias[:, t0:t0 + t_local, 0].T).astype(np.float32))
        pfx = np.zeros((n_chunks + 2, n_chunks), np.float32)
        for k in range(n_chunks):
            pfx[k, k + 1:] = 1.0
        if half == 1:
            pfx[n_chunks, :] = 1.0  # partner (pair rank 0) total
        in_maps.append({
            "xT": xT, "waT": waT, "wpTn": wpTn, "ba": ba_r, "bp": bp_r,
            "tb": tb, "db64": db64, "pfx": pfx, "onesr": onesr, "onesb": onesb,
        })
    return in_maps


def kernel(x, Wa, ba, Wp, bp, temp, denom_bias):
    x = np.asarray(x)
    nc = _get_nc()
    in_maps = make_in_maps(np.asarray(x), np.asarray(Wa), np.asarray(ba),
                           np.asarray(Wp), np.asarray(bp), np.asarray(temp),
                           np.asarray(denom_bias))
    from concourse import bass_utils
    res = bass_utils.run_bass_kernel_spmd(nc, in_maps, core_ids=list(range(N_CORES)))
    out = np.empty((B, T, C), np.float32)
    for i in range(N_CORES):
        b, half = i // 2, i % 2
        out[b, half * T_LOCAL:(half + 1) * T_LOCAL, :] = res.results[i]["out"]
    return out


# revision 23
# speedup vs baseline: 1.0208x; 1.0208x over previous
"""TSSA causal self-attention Bass kernel for 8 TRN2 NeuronCores.

Math (per batch b):
    w      = x @ Wa.T + ba                  # (T, C) -> heads (H, T, D)
    wsq    = w * w
    denom  = cumsum_T(wsq)                  # inclusive
    tmp    = (sum_d(wsq / denom) + D*db) * temp          # (T, H)
    Pi     = softmax_h(tmp)                 # (T, H)
    cumA   = cumsum_T(wsq * Pi)
    cumPi  = cumsum_T(Pi) + 1e-8
    y      = -(w * Pi) * (1 / (1 + cumA / cumPi))
           = (w * Pi * cumPi) * (-1 / (cumA + cumPi))
    out    = y @ Wp.T + bp

Sharding: core i -> (batch b = i//2, T-half = i%2).  Each core runs the full
pipeline on its (b, T/2) slice in [t-on-partitions, c-free] layout, chunked by
128 t-rows.  Cumsums over T are triangular matmuls on the PE plus per-chunk
carry rows; the cross-half carries travel via two tiny pairwise AllGathers.
"""

import numpy as np
import ml_dtypes

B, T, C, H, D = 4, 4096, 1024, 16, 64
N_CORES = 8
P = 128
T_LOCAL = T // 2

F32 = None  # filled on bass import
BF16 = None

_BUILD_CACHE = {}


def _build(t_local, n_groups, use_bf16=True, fake_comm=False):
    """Build the SPMD Bass program. n_groups = number of core pairs."""
    import concourse.bass as bass
    import concourse.bacc as bacc
    import concourse.mybir as mybir
    from concourse import tile

    dt = mybir.dt
    f32, bf16, f32r = dt.float32, dt.bfloat16, dt.float32r
    AF = mybir.ActivationFunctionType
    OP = mybir.AluOpType

    n_chunks = t_local // P
    n_cores = 2 * n_groups
    NH = n_chunks  # alias

    wdt = bf16 if use_bf16 else f32  # dtype for wsp store + proj2 path
    npwdt = ml_dtypes.bfloat16 if use_bf16 else np.float32

    nc = bacc.Bacc(None, target_bir_lowering=False, debug=False)

    # ---------------- I/O ----------------
    xT = nc.dram_tensor("xT", [C, t_local], f32r, kind="ExternalInput")
    waT = nc.dram_tensor("waT", [C, C], f32r, kind="ExternalInput")
    wpTn = nc.dram_tensor("wpTn", [C, C], wdt, kind="ExternalInput")
    ba_in = nc.dram_tensor("ba", [1, C], f32r, kind="ExternalInput")
    onesr_in = nc.dram_tensor("onesr", [1, P], f32r, kind="ExternalInput")
    onesb_in = nc.dram_tensor("onesb", [1, P], wdt, kind="ExternalInput")
    bp_in = nc.dram_tensor("bp", [1, C], wdt, kind="ExternalInput")
    tb_in = nc.dram_tensor("tb", [P, H], f32, kind="ExternalInput")
    db_in = nc.dram_tensor("db64", [t_local, H], f32, kind="ExternalInput")
    pfx_in = nc.dram_tensor("pfx", [NH + 2, NH], f32, kind="ExternalInput")
    out = nc.dram_tensor("out", [t_local, C], f32, kind="ExternalOutput")

    # constants baked into the NEFF
    ut_np = np.triu(np.ones((P, P), np.float32))
    ut_c = nc.inline_tensor(ut_np, "ut_c")
    utb_c = nc.inline_tensor(ut_np.astype(npwdt), "utb_c")
    ones_np = np.ones((P, P), np.float32)
    ones_c = nc.inline_tensor(ones_np, "ones_c")
    eye_np = np.eye(P, dtype=np.float32)
    eye_c = nc.inline_tensor(eye_np, "eye_c")
    eyeb_c = nc.inline_tensor(eye_np.astype(npwdt), "eyeb_c")
    bm_np = np.zeros((H, C), np.float32)
    for h in range(H):
        bm_np[h, h * D:(h + 1) * D] = 1.0
    bm_c = nc.inline_tensor(bm_np, "bm_c")
    bmb_c = nc.inline_tensor(bm_np.astype(npwdt), "bmb_c")
    # one-hot chunk selectors: oneh[:, j, m] = (m == j)
    oneh_np = np.zeros((P, NH, NH), np.float32)
    for j in range(NH):
        oneh_np[:, j, j] = 1.0
    oneh_c = nc.inline_tensor(oneh_np.reshape(P, NH * NH), "oneh_c")
    onehb_c = nc.inline_tensor(oneh_np.reshape(P, NH * NH).astype(npwdt),
                               "onehb_c")

    # internal DRAM for collectives
    cc1_in = nc.dram_tensor("cc1_in", [1, C], f32, kind="Internal")
    cc1_out = nc.dram_tensor("cc1_out", [2, C], f32, kind="Internal")
    cc2_in = nc.dram_tensor("cc2_in", [1, C + H], f32, kind="Internal")
    cc2_out = nc.dram_tensor("cc2_out", [2, C + H], f32, kind="Internal")
    carr_d = nc.dram_tensor("carr_d", [t_local // P, C], wdt, kind="Internal")
    carrA_d = nc.dram_tensor("carrA_d", [t_local // P, C], wdt, kind="Internal")
    carrPi_d = nc.dram_tensor("carrPi_d", [t_local // P, H], f32, kind="Internal")
    rg1 = [[2 * g, 2 * g + 1] for g in range(n_groups)]

    def r(ap):
        return ap.bitcast(f32r)

    with tile.TileContext(nc) as tc:
        with (
            tc.tile_pool(name="const", bufs=1) as cpool,
            tc.tile_pool(name="persist", bufs=1) as pp,
            tc.tile_pool(name="wmat", bufs=1) as wm,
        ):
            # ---- constants to SBUF ----
            ut_s = cpool.tile([P, P], f32, tag="ut")
            nc.sync.dma_start(ut_s[:, :], ut_c.ap())
            utb_s = cpool.tile([P, P], wdt, tag="utb")
            nc.sync.dma_start(utb_s[:, :], utb_c.ap())
            ones_s = cpool.tile([P, P], f32, tag="ones")
            nc.sync.dma_start(ones_s[:, :], ones_c.ap())
            eye_s = cpool.tile([P, P], f32, tag="eye")
            nc.sync.dma_start(eye_s[:, :], eye_c.ap())
            eyeb_s = cpool.tile([P, P], wdt, tag="eyeb")
            nc.sync.dma_start(eyeb_s[:, :], eyeb_c.ap())
            bmb_s = cpool.tile([H, C], wdt, tag="bmb")
            nc.sync.dma_start(bmb_s[:, :], bmb_c.ap())
            ba_s = cpool.tile([1, C], f32r, tag="ba")
            nc.sync.dma_start(ba_s[:, :], ba_in.ap())
            bp_s = cpool.tile([1, C], wdt, tag="bp")
            nc.sync.dma_start(bp_s[:, :], bp_in.ap())
            tb_s = cpool.tile([P, H], f32, tag="tb")
            nc.sync.dma_start(tb_s[:, :], tb_in.ap())
            onesr_s = cpool.tile([1, P], f32r, tag="onesr")
            nc.sync.dma_start(onesr_s[:, :], onesr_in.ap())
            onesb_s = cpool.tile([1, P], wdt, tag="onesb")
            nc.sync.dma_start(onesb_s[:, :], onesb_in.ap())
            db_s = cpool.tile([P, NH, H], f32, tag="db")
            nc.sync.dma_start(db_s[:, :, :],
                              db_in.ap().rearrange("(j p) h -> p j h", p=P))
            pfx_s = cpool.tile([NH + 2, NH], f32, tag="pfx")
            nc.sync.dma_start(pfx_s[:, :], pfx_in.ap())
            oneh_s = cpool.tile([P, NH, NH], f32, tag="oneh")
            nc.sync.dma_start(
                oneh_s[:, :, :],
                oneh_c.ap().rearrange("p (j m) -> p j m", j=NH))
            onehb_s = cpool.tile([P, NH, NH], wdt, tag="onehb")
            nc.sync.dma_start(
                onehb_s[:, :, :],
                onehb_c.ap().rearrange("p (j m) -> p j m", j=NH))

            # ---- persistent stores ----
            w_st = pp.tile([P, NH, C], f32, tag="w_st")
            wsp_st = pp.tile([P, NH, C], wdt, tag="wsp_st")
            pi_st = pp.tile([P, NH, H], f32, tag="pi_st")
            s_tbl = pp.tile([NH + 2, C], f32, tag="s_tbl")
            swsp_tbl = pp.tile([NH + 2, C], f32, tag="swsp_tbl")
            spi_tbl = pp.tile([NH + 2, H], f32, tag="spi_tbl")

            wa_s = wm.tile([P, 8, C], f32r, tag="wa")
            nc.sync.dma_start(wa_s[:, :, :],
                              waT.ap().rearrange("(a p) n -> p a n", p=P))

            xT_r = xT.ap().rearrange("(a p) t -> p a t", p=P)

            # ================= phase 1: proj1, store w, S_wsq rows ========
            with (
                tc.tile_pool(name="xt", bufs=3) as xt_pool,
                tc.tile_pool(name="sq1", bufs=2) as sq_pool,
                tc.tile_pool(name="pw", bufs=2, space="PSUM") as pw_pool,
                tc.tile_pool(name="pS", bufs=1, space="PSUM") as pS_pool,
            ):
                pS = pS_pool.tile([NH, C], f32, tag="pS")
                for j in range(n_chunks):
                    xt = xt_pool.tile([P, 8, P], f32r, tag="xt")
                    nc.sync.dma_start(xt[:, :, :], xT_r[:, :, j * P:(j + 1) * P])
                    pw = pw_pool.tile([P, C], f32, tag="pw")
                    for hh in range(2):
                        o = pw[:, hh * 512:(hh + 1) * 512]
                        for a in range(8):
                            nc.tensor.matmul(
                                o, xt[:, a, :],
                                wa_s[:, a, hh * 512:(hh + 1) * 512],
                                start=(a == 0), stop=False)
                        nc.tensor.matmul(
                            o, onesr_s[0:1, :],
                            ba_s[0:1, hh * 512:(hh + 1) * 512],
                            start=False, stop=True)
                    # drain: w copy (DVE) + square (ACT)
                    nc.vector.tensor_copy(w_st[:, j, :], pw[:, :])
                    sq = sq_pool.tile([P, C], wdt, tag="sq1")
                    nc.scalar.activation(sq[:, :], pw[:, :], AF.Square)
                    # S_wsq row j (one-hot selector accumulates into row j)
                    for hh in range(2):
                        nc.tensor.matmul(
                            pS[0:NH, hh * 512:(hh + 1) * 512],
                            onehb_s[:, j, :],
                            sq[:, hh * 512:(hh + 1) * 512],
                            start=(j == 0), stop=(j == n_chunks - 1))
                # export S rows + total
                nc.vector.tensor_copy(s_tbl[0:NH, :], pS[:, :])
                pt = pw_pool.tile([P, C], f32, tag="pw")
                for hh in range(2):
                    nc.tensor.matmul(pt[0:1, hh * 512:(hh + 1) * 512],
                                     ones_s[0:NH, 0:1],
                                     s_tbl[0:NH, hh * 512:(hh + 1) * 512],
                                     start=True, stop=True)
                cc1_stage = sq_pool.tile([1, C], f32, tag="cc1s")
                nc.vector.tensor_copy(cc1_stage[:, :], pt[0:1, :])
                nc.sync.dma_start(cc1_in.ap(), cc1_stage[:, :])

            # ---- collective 1 + denom carry table ----
            if fake_comm:
                nc.sync.dma_start(cc1_out.ap()[0:1, :], cc1_in.ap())
                nc.sync.dma_start(cc1_out.ap()[1:2, :], cc1_in.ap())
            else:
                nc.gpsimd.collective_compute(
                    "AllGather", mybir.AluOpType.bypass,
                    replica_groups=rg1,
                    ins=[cc1_in.ap().opt()],
                    outs=[cc1_out.ap().opt()])
            nc.sync.dma_start(s_tbl[NH:NH + 2, :], cc1_out.ap())

            with (
                tc.tile_pool(name="pc", bufs=1, space="PSUM") as pc_pool,
                tc.tile_pool(name="pcs", bufs=1) as pcs_pool,
            ):
                pc = pc_pool.tile([NH, C], f32, tag="pc")
                for hh in range(2):
                    nc.tensor.matmul(pc[:, hh * 512:(hh + 1) * 512],
                                     pfx_s[:, :],
                                     s_tbl[:, hh * 512:(hh + 1) * 512],
                                     start=True, stop=True)
                pcs = pcs_pool.tile([NH, C], wdt, tag="pcs")
                nc.vector.tensor_copy(pcs[:, :], pc[:, :])
                nc.sync.dma_start(carr_d.ap(), pcs[:, :])

            # ============ phase 2: denom, softmax, wsp, S rows ===========
            with (
                tc.tile_pool(name="sq2", bufs=2) as sq_pool,
                tc.tile_pool(name="sqb2", bufs=2) as sqb_pool,
                tc.tile_pool(name="rd2", bufs=2) as rd_pool,
                tc.tile_pool(name="wn2", bufs=2) as wn_pool,
                tc.tile_pool(name="t2a", bufs=3) as tiny_pool,
                tc.tile_pool(name="ct2", bufs=3) as ct_pool,
                tc.tile_pool(name="pd", bufs=2, space="PSUM") as pd_pool,
                tc.tile_pool(name="pSP", bufs=1, space="PSUM") as pSP_pool,
                tc.tile_pool(name="pSW", bufs=1, space="PSUM") as pSW_pool,
            ):
                pSPI = pSP_pool.tile([NH, H], f32, tag="pSPI")
                pSW = pSW_pool.tile([NH, C], f32, tag="pSW")
                for j in range(n_chunks):
                    sq = sq_pool.tile([P, C], f32, tag="sq2")
                    nc.scalar.activation(sq[:, :], w_st[:, j, :], AF.Square)
                    sqb = sqb_pool.tile([P, C], wdt, tag="sqb2")
                    nc.scalar.activation(sqb[:, :], w_st[:, j, :], AF.Square)
                    ct = ct_pool.tile([1, C], wdt, tag="ct2")
                    nc.sync.dma_start(ct[0:1, :], carr_d.ap()[j:j + 1, :])
                    pd = pd_pool.tile([P, C], f32, tag="pd")
                    for hh in range(2):
                        o = pd[:, hh * 512:(hh + 1) * 512]
                        nc.tensor.matmul(o, utb_s[:, :],
                                         sqb[:, hh * 512:(hh + 1) * 512],
                                         start=True, stop=False)
                        nc.tensor.matmul(o, onesb_s[0:1, :],
                                         ct[0:1, hh * 512:(hh + 1) * 512],
                                         start=False, stop=True)
                    rd = rd_pool.tile([P, C], f32, tag="rd2")
                    nc.vector.reciprocal_approx_fast(rd[:, :], pd[:, :])
                    wn = wn_pool.tile([P, C], f32, tag="wn2")
                    nc.vector.tensor_mul(wn[:, :], sq[:, :], rd[:, :])
                    red = tiny_pool.tile([P, H], f32, tag="red")
                    nc.vector.tensor_reduce(
                        red[:, :], wn[:, :].rearrange("p (h d) -> p h d", d=D),
                        axis=mybir.AxisListType.X, op=OP.add)
                    t1 = tiny_pool.tile([P, H], f32, tag="t1")
                    nc.vector.tensor_add(t1[:, :], red[:, :], db_s[:, j, :])
                    tmpj = tiny_pool.tile([P, H], f32, tag="tmpj")
                    nc.vector.tensor_mul(tmpj[:, :], t1[:, :], tb_s[:, :])
                    nm = tiny_pool.tile([P, 1], f32, tag="nm")
                    nc.vector.tensor_reduce(
                        nm[:, :], tmpj[:, :],
                        axis=mybir.AxisListType.X, op=OP.max, negate=True)
                    es = tiny_pool.tile([P, H], f32, tag="es")
                    nc.scalar.activation(es[:, :], tmpj[:, :], AF.Exp,
                                         bias=nm[:, :], scale=1.0)
                    rs = tiny_pool.tile([P, 1], f32, tag="rs")
                    nc.vector.tensor_reduce(rs[:, :], es[:, :],
                                            axis=mybir.AxisListType.X, op=OP.add)
                    rr = tiny_pool.tile([P, 1], f32, tag="rr")
                    nc.vector.reciprocal(rr[:, :], rs[:, :])
                    nc.vector.tensor_scalar_mul(pi_st[:, j, :], es[:, :],
                                                rr[:, :])
                    nc.vector.tensor_mul(
                        wsp_st[:, j, :], sq[:, :],
                        pi_st[:, j, :].rearrange("p (h o) -> p h o", o=1)
                        .to_broadcast((P, H, D)))
                    nc.tensor.matmul(pSPI[0:NH, :], oneh_s[:, j, :],
                                     pi_st[:, j, :],
                                     start=(j == 0), stop=(j == n_chunks - 1))
                    for hh in range(2):
                        nc.tensor.matmul(
                            pSW[0:NH, hh * 512:(hh + 1) * 512],
                            onehb_s[:, j, :],
                            wsp_st[:, j, hh * 512:(hh + 1) * 512],
                            start=(j == 0), stop=(j == n_chunks - 1))
                nc.vector.tensor_copy(swsp_tbl[0:NH, :], pSW[:, :])
                nc.vector.tensor_copy(spi_tbl[0:NH, :], pSPI[:, :])
                # totals
                pt2 = pSW_pool.tile([NH, C], f32, tag="pSW")
                for hh in range(2):
                    nc.tensor.matmul(pt2[0:1, hh * 512:(hh + 1) * 512],
                                     ones_s[0:NH, 0:1],
                                     swsp_tbl[0:NH, hh * 512:(hh + 1) * 512],
                                     start=True, stop=True)
                pt3 = pSP_pool.tile([NH, H], f32, tag="pSPI")
                nc.tensor.matmul(pt3[0:1, 0:H], ones_s[0:NH, 0:1],
                                 spi_tbl[0:NH, :], start=True, stop=True)
                cc2_stage = sq_pool.tile([1, C + H], f32, tag="cc2s")
                nc.vector.tensor_copy(cc2_stage[:, 0:C], pt2[0:1, :])
                nc.vector.tensor_copy(cc2_stage[:, C:C + H], pt3[0:1, 0:H])
                nc.sync.dma_start(cc2_in.ap(), cc2_stage[:, :])

            # ---- collective 2 + dots carry tables ----
            if fake_comm:
                nc.sync.dma_start(cc2_out.ap()[0:1, :], cc2_in.ap())
                nc.sync.dma_start(cc2_out.ap()[1:2, :], cc2_in.ap())
            else:
                nc.gpsimd.collective_compute(
                    "AllGather", mybir.AluOpType.bypass,
                    replica_groups=rg1,
                    ins=[cc2_in.ap().opt()],
                    outs=[cc2_out.ap().opt()])
            nc.sync.dma_start(swsp_tbl[NH:NH + 2, :], cc2_out.ap()[:, 0:C])
            nc.sync.dma_start(spi_tbl[NH:NH + 2, :], cc2_out.ap()[:, C:C + H])

            with (
                tc.tile_pool(name="pc2", bufs=1, space="PSUM") as pc_pool,
                tc.tile_pool(name="pcs2", bufs=1) as pcs_pool,
            ):
                pc = pc_pool.tile([NH, C], f32, tag="pc2")
                for hh in range(2):
                    nc.tensor.matmul(pc[:, hh * 512:(hh + 1) * 512],
                                     pfx_s[:, :],
                                     swsp_tbl[:, hh * 512:(hh + 1) * 512],
                                     start=True, stop=True)
                pcs = pcs_pool.tile([NH, C], wdt, tag="pcs2")
                nc.vector.tensor_copy(pcs[:, :], pc[:, :])
                nc.sync.dma_start(carrA_d.ap(), pcs[:, :])
                pc3 = pc_pool.tile([NH, C], f32, tag="pc2")
                nc.tensor.matmul(pc3[:, 0:H], pfx_s[:, :], spi_tbl[:, :],
                                 start=True, stop=True)
                pcs3 = pcs_pool.tile([NH, H], f32, tag="pcs3")
                nc.vector.tensor_copy(pcs3[:, :], pc3[:, 0:H])
                nc.sync.dma_start(carrPi_d.ap(), pcs3[:, :])

            # ================= phase 3: dots, attn, y, proj2 =============
            wp_s = wm.tile([P, 8, C], wdt, tag="wa")
            nc.sync.dma_start(wp_s[:, :, :],
                              wpTn.ap().rearrange("(a p) n -> p a n", p=P))
            with (
                tc.tile_pool(name="t3", bufs=2) as tiny_pool,
                tc.tile_pool(name="rd3", bufs=2) as rd_pool,
                tc.tile_pool(name="yy", bufs=2) as y_pool,
                tc.tile_pool(name="yt", bufs=2) as yt_pool,
                tc.tile_pool(name="ost", bufs=2) as o_pool,
                tc.tile_pool(name="psm", bufs=2, space="PSUM") as psm_pool,
                tc.tile_pool(name="pD", bufs=2, space="PSUM") as pD_pool,
                tc.tile_pool(name="pyt", bufs=1, space="PSUM") as pyt_pool,
                tc.tile_pool(name="po", bufs=2, space="PSUM") as po_pool,
            ):
                for j in range(n_chunks):
                    # cumPi for this chunk
                    ctp = tiny_pool.tile([1, H], f32, tag="ctp")
                    nc.sync.dma_start(ctp[0:1, :], carrPi_d.ap()[j:j + 1, :])
                    cta = tiny_pool.tile([1, C], wdt, tag="cta")
                    nc.sync.dma_start(cta[0:1, :], carrA_d.ap()[j:j + 1, :])
                    ps = psm_pool.tile([P, 512], f32, tag="psm")
                    nc.tensor.matmul(ps[:, 0:H], ut_s[:, :], pi_st[:, j, :],
                                     start=True, stop=False)
                    nc.tensor.matmul(ps[:, 0:H], ones_s[0:1, :],
                                     ctp[0:1, :],
                                     start=False, stop=True)
                    cpe = tiny_pool.tile([P, H], f32, tag="cpe")
                    nc.vector.tensor_scalar_add(cpe[:, :], ps[:, 0:H], 1e-8)
                    # transpose cpe -> [H, P]
                    nc.tensor.transpose(ps[0:H, 128:256], cpe[:, :],
                                        eye_s[:, :])
                    cpt = tiny_pool.tile([H, P], wdt, tag="cpt")
                    nc.vector.tensor_copy(cpt[:, :], ps[0:H, 128:256])
                    # D = cumA + bcast(cumPi_e)
                    rd = rd_pool.tile([P, C], f32, tag="rd3")
                    for hh in range(2):
                        pD = pD_pool.tile([P, 512], f32, tag="pD")
                        nc.tensor.matmul(pD[:, :], utb_s[:, :],
                                         wsp_st[:, j, hh * 512:(hh + 1) * 512],
                                         start=True, stop=False)
                        nc.tensor.matmul(pD[:, :], onesb_s[0:1, :],
                                         cta[0:1, hh * 512:(hh + 1) * 512],
                                         start=False, stop=False)
                        nc.tensor.matmul(pD[:, :], cpt[:, :],
                                         bmb_s[:, hh * 512:(hh + 1) * 512],
                                         start=False, stop=True)
                        nc.vector.reciprocal_approx_fast(
                            rd[:, hh * 512:(hh + 1) * 512], pD[:, :])
                    # g = Pi * cumPi_e
                    g = tiny_pool.tile([P, H], f32, tag="g")
                    nc.vector.tensor_mul(g[:, :], pi_st[:, j, :], cpe[:, :])
                    # y = (w * rd) * g_bcast   (positive; sign folded into wpTn)
                    t1 = y_pool.tile([P, C], f32, tag="t1f")
                    nc.vector.tensor_mul(t1[:, :], w_st[:, j, :], rd[:, :])
                    y = y_pool.tile([P, C], wdt, tag="ybf")
                    nc.vector.tensor_mul(
                        y[:, :], t1[:, :],
                        g[:, :].rearrange("p (h o) -> p h o", o=1).to_broadcast((P, H, D)))
                    # transpose y -> yT (8 PE transposes via 2 psum rounds)
                    yt = yt_pool.tile([P, 8, P], wdt, tag="yt")
                    for rnd in range(2):
                        pyt = pyt_pool.tile([P, 512], wdt, tag="pyt")
                        for i in range(4):
                            a = rnd * 4 + i
                            nc.tensor.transpose(
                                pyt[:, i * P:(i + 1) * P],
                                y[:, a * P:(a + 1) * P], eyeb_s[:, :])
                        nc.scalar.copy(yt[:, rnd * 4:rnd * 4 + 4, :],
                                       pyt[:, :])
                    # proj2
                    for hh in range(2):
                        po = po_pool.tile([P, 512], f32, tag="po")
                        for a in range(8):
                            nc.tensor.matmul(
                                po[:, :], yt[:, a, :],
                                wp_s[:, a, hh * 512:(hh + 1) * 512],
                                start=(a == 0), stop=False)
                        nc.tensor.matmul(po[:, :], onesb_s[0:1, :],
                                         bp_s[0:1, hh * 512:(hh + 1) * 512],
                                         start=False, stop=True)
                        ost = o_pool.tile([P, 512], f32, tag="ost")
                        nc.scalar.copy(ost[:, :], po[:, :])
                        nc.sync.dma_start(
                            out.ap()[j * P:(j + 1) * P,
                                     hh * 512:(hh + 1) * 512], ost[:, :])

    nc.finalize()
    return nc


def _get_nc(t_local=T_LOCAL, n_groups=4, use_bf16=True, fake_comm=False):
    key = (t_local, n_groups, use_bf16, fake_comm)
    if key not in _BUILD_CACHE:
        _BUILD_CACHE[key] = _build(t_local, n_groups, use_bf16, fake_comm)
    return _BUILD_CACHE[key]


def make_in_maps(x, Wa, ba, Wp, bp, temp, denom_bias, t_local=T_LOCAL,
                 n_groups=4, use_bf16=True):
    """Host-side sharding: core i -> (b=i//2, half=i%2)."""
    npwdt = ml_dtypes.bfloat16 if use_bf16 else np.float32
    n_chunks = t_local // P
    waT = np.ascontiguousarray(Wa.T.astype(np.float32))
    onesr = np.ones((1, P), np.float32)
    onesb = np.ones((1, P), npwdt)
    wpTn = np.ascontiguousarray((-Wp.T).astype(np.float32)).astype(npwdt)
    ba_r = np.ascontiguousarray(ba.reshape(1, C).astype(np.float32))
    bp_r = np.ascontiguousarray(bp.reshape(1, C).astype(np.float32)).astype(npwdt)
    tb = np.broadcast_to(temp.reshape(1, H), (P, H)).astype(np.float32)
    tb = np.ascontiguousarray(tb)
    in_maps = []
    for i in range(2 * n_groups):
        b, half = i // 2, i % 2
        t0 = half * t_local
        xT = np.ascontiguousarray(x[b, t0:t0 + t_local, :].T.astype(np.float32))
        db64 = np.ascontiguousarray(
            (D * denom_bias[:, t0:t0 + t_local, 0].T).astype(np.float32))
        pfx = np.zeros((n_chunks + 2, n_chunks), np.float32)
        for k in range(n_chunks):
            pfx[k, k + 1:] = 1.0
        if half == 1:
            pfx[n_chunks, :] = 1.0  # partner (pair rank 0) total
        in_maps.append({
            "xT": xT, "waT": waT, "wpTn": wpTn, "ba": ba_r, "bp": bp_r,
            "tb": tb, "db64": db64, "pfx": pfx, "onesr": onesr, "onesb": onesb,
        })
    return in_maps


def kernel(x, Wa, ba, Wp, bp, temp, denom_bias):
    x = np.asarray(x)
    nc = _get_nc()
    in_maps = make_in_maps(np.asarray(x), np.asarray(Wa), np.asarray(ba),
                           np.asarray(Wp), np.asarray(bp), np.asarray(temp),
                           np.asarray(denom_bias))
    from concourse import bass_utils
    res = bass_utils.run_bass_kernel_spmd(nc, in_maps, core_ids=list(range(N_CORES)))
    out = np.empty((B, T, C), np.float32)
    for i in range(N_CORES):
        b, half = i // 2, i % 2
        out[b, half * T_LOCAL:(half + 1) * T_LOCAL, :] = res.results[i]["out"]
    return out
